# revision 11
# baseline (speedup 1.0000x reference)
"""BrickTube kernel for 8x Trainium2 NeuronCores.

The reference "BrickTube" module applies 80 tiny (2,2,2,2) gate cores to a
[B, 1024] state tensor. Every gate application is linear in x and
INPUT_DIM == BINDIM == OUTPUT_DIM == 1024, so the whole module collapses to

    out = x @ W,   W[i, :] = circuit(e_i)  (1024 x 1024)

W is built exactly on the host in float64 from `cores` (cheap: 80 small
tensordots), then the device runs a batch-sharded dense matmul:
each of the 8 cores computes y_c^T = W^T @ x_c^T for its 4096-row shard of x,
with bf16 operands and fp32 PSUM accumulation.
"""

import math

import ml_dtypes
import numpy as np

# ---- problem constants (hardcoded per contract) ----
B = 32768
D = 1024
N_CORES = 8
NPC = B // N_CORES  # 4096 batch rows per core

BOND = 2
Q = 10
N_LAYERS = 8
PAIRS1 = [(i, i + 1) for i in range(0, Q, 2)]
PAIRS2 = [(i, (i + 1) % Q) for i in range(1, Q, 2)]
HALF = Q // 2

BF16 = ml_dtypes.bfloat16


def build_w(cores: np.ndarray) -> np.ndarray:
    """Collapse the 80-gate circuit into W [1024, 1024] (float64),
    with out_row = x_row @ W."""
    c = cores.astype(np.float64)
    s = np.eye(D, dtype=np.float64).reshape((D,) + (BOND,) * Q)
    for layer in range(N_LAYERS):
        base = layer * Q
        for g, (i, j) in enumerate(PAIRS1):
            s = np.tensordot(s, c[base + g], axes=((i + 1, j + 1), (0, 1)))
            s = np.moveaxis(s, (-2, -1), (i + 1, j + 1))
        for g, (i, j) in enumerate(PAIRS2):
            s = np.tensordot(s, c[base + HALF + g], axes=((i + 1, j + 1), (0, 1)))
            s = np.moveaxis(s, (-2, -1), (i + 1, j + 1))
    return s.reshape(D, D)


_NC_CACHE = None


def _build_bass():
    """Device program (identical on all 8 cores):
      inputs:  xt [1024, 4096] bf16  (x-shard transposed: xt[k, n])
               w  [1024, 1024] bf16  (W[k, m], k = contraction)
      output:  yt [1024, 4096] fp32  (y-shard transposed: yt[m, n])
    yt[m, n] = sum_k w[k, m] * xt[k, n]
    """
    global _NC_CACHE
    if _NC_CACHE is not None:
        return _NC_CACHE

    import concourse.bacc as bacc
    import concourse.mybir as mybir
    import concourse.tile as tile

    KC = D // 128  # 8 contraction chunks
    MC = D // 128  # 8 output-row chunks
    JC = NPC // 512  # 8 batch column chunks

    nc = bacc.Bacc("TRN2")
    xt = nc.dram_tensor("xt", [D, NPC], mybir.dt.bfloat16, kind="ExternalInput")
    w = nc.dram_tensor("w", [D, D], mybir.dt.bfloat16, kind="ExternalInput")
    yt = nc.dram_tensor("yt", [D, NPC], mybir.dt.float32, kind="ExternalOutput")

    with tile.TileContext(nc) as tc:
        with (
            tc.tile_pool(name="xpool", bufs=1) as xpool,
            tc.tile_pool(name="wpool", bufs=1) as wpool,
            tc.tile_pool(name="opool", bufs=2) as opool,
            tc.tile_pool(name="psum", bufs=1, space="PSUM") as ppool,
        ):
            # per-k-chunk tiles so compute can start before all input DMAs land
            wk = []
            xk = []
            for k in range(KC):
                wt = wpool.tile(
                    [128, D], mybir.dt.bfloat16, name=f"wk{k}", tag=f"w{k}"
                )
                nc.sync.dma_start(wt[:], w[k * 128 : (k + 1) * 128, :])
                wk.append(wt)
                xtile = xpool.tile(
                    [128, NPC], mybir.dt.bfloat16, name=f"xk{k}", tag=f"x{k}"
                )
                nc.sync.dma_start(xtile[:], xt[k * 128 : (k + 1) * 128, :])
                xk.append(xtile)

            for m in range(MC):
                psums = [
                    ppool.tile(
                        [128, 512], mybir.dt.float32, name=f"ps{j}", tag=f"ps{j}"
                    )
                    for j in range(JC)
                ]
                for k in range(KC):
                    lhsT = wk[k][:, m * 128 : (m + 1) * 128]
                    for j in range(JC):
                        nc.tensor.matmul(
                            psums[j][:],
                            lhsT,
                            xk[k][:, j * 512 : (j + 1) * 512],
                            start=(k == 0),
                            stop=(k == KC - 1),
                        )
                for j in range(JC):
                    osb = opool.tile(
                        [128, 512], mybir.dt.float32, name=f"osb{j}", tag=f"osb{j}"
                    )
                    nc.vector.tensor_copy(osb[:], psums[j][:])
                    nc.sync.dma_start(
                        yt[m * 128 : (m + 1) * 128, j * 512 : (j + 1) * 512], osb[:]
                    )

    nc.compile()
    _NC_CACHE = nc
    return nc


def _run(x: np.ndarray, cores: np.ndarray, trace: bool = False):
    from concourse.bass_utils import run_bass_kernel_spmd

    W = build_w(cores)
    wb = np.ascontiguousarray(W.astype(np.float32).astype(BF16))

    xb = x.astype(BF16)
    in_maps = []
    for c in range(N_CORES):
        xt_c = np.ascontiguousarray(xb[c * NPC : (c + 1) * NPC, :].T)
        in_maps.append({"xt": xt_c, "w": wb})

    nc = _build_bass()
    res = run_bass_kernel_spmd(
        nc, in_maps, core_ids=list(range(N_CORES)), trace=trace
    )

    y = np.empty((B, D), dtype=np.float32)
    for c in range(N_CORES):
        y[c * NPC : (c + 1) * NPC, :] = res.results[c]["yt"].T
    return y, res


def kernel(x: np.ndarray, cores: np.ndarray) -> np.ndarray:
    y, _ = _run(x, cores, trace=False)
    return y


# revision 13
# speedup vs baseline: 1.0741x; 1.0741x over previous
"""BrickTube kernel for 8x Trainium2 NeuronCores.

The reference "BrickTube" module applies 80 tiny (2,2,2,2) gate cores to a
[B, 1024] state tensor. Every gate application is linear in x and
INPUT_DIM == BINDIM == OUTPUT_DIM == 1024, so the whole module collapses to

    out = x @ W,   W[i, :] = circuit(e_i)  (1024 x 1024)

W is built exactly on the host in float64 from `cores` (cheap: 80 small
tensordots), then the device runs a batch-sharded dense matmul:
each of the 8 cores computes y_c^T = W^T @ x_c^T for its 4096-row shard of x,
with fp16 operands (same PE rate as bf16, 8x the mantissa accuracy) and fp32
PSUM accumulation.

Device kernel structure (per core):
  - j-outer loop over 8 batch chunks of 512 so input DMA delivery stays ahead
    of PE consumption (each j-chunk is 1MB of x^T vs ~14us of matmuls).
  - 8 PSUM banks (one per output-row chunk m), accumulated over the 8
    contraction chunks k; drained bank-by-bank while the next j streams.
  - ~3.5us of tiny warmup matmuls on zeros so the PE HAM clock-gate is at
    full rate (2.4 GHz) by the time real data lands.
  - x-chunk DMAs on the Sync HWDGE ring, w on the Scalar ring (parallel
    issue); PSUM drains alternate between Vector and Scalar engines.
"""

import math

import ml_dtypes
import numpy as np

# ---- problem constants (hardcoded per contract) ----
B = 32768
D = 1024
N_CORES = 8
NPC = B // N_CORES  # 4096 batch rows per core

BOND = 2
Q = 10
N_LAYERS = 8
PAIRS1 = [(i, i + 1) for i in range(0, Q, 2)]
PAIRS2 = [(i, (i + 1) % Q) for i in range(1, Q, 2)]
HALF = Q // 2


def build_w(cores: np.ndarray) -> np.ndarray:
    """Collapse the 80-gate circuit into W [1024, 1024] (float64),
    with out_row = x_row @ W."""
    c = cores.astype(np.float64)
    s = np.eye(D, dtype=np.float64).reshape((D,) + (BOND,) * Q)
    for layer in range(N_LAYERS):
        base = layer * Q
        for g, (i, j) in enumerate(PAIRS1):
            s = np.tensordot(s, c[base + g], axes=((i + 1, j + 1), (0, 1)))
            s = np.moveaxis(s, (-2, -1), (i + 1, j + 1))
        for g, (i, j) in enumerate(PAIRS2):
            s = np.tensordot(s, c[base + HALF + g], axes=((i + 1, j + 1), (0, 1)))
            s = np.moveaxis(s, (-2, -1), (i + 1, j + 1))
    return s.reshape(D, D)


_NC_CACHE = None


def _build_bass():
    """Device program (identical on all 8 cores):
      inputs:  xt [1024, 4096] fp16  (x-shard transposed: xt[k, n])
               w  [1024, 1024] fp16  (W[k, m], k = contraction)
      output:  yt [1024, 4096] fp32  (y-shard transposed: yt[m, n])
    yt[m, n] = sum_k w[k, m] * xt[k, n]
    """
    global _NC_CACHE
    if _NC_CACHE is not None:
        return _NC_CACHE

    import concourse.bacc as bacc
    import concourse.mybir as mybir
    import concourse.tile as tile

    KC = D // 128  # 8 contraction chunks
    MC = D // 128  # 8 output-row chunks
    JC = NPC // 512  # 8 batch column chunks
    F16 = mybir.dt.float16
    F32 = mybir.dt.float32

    nc = bacc.Bacc("TRN2")
    xt = nc.dram_tensor("xt", [D, NPC], F16, kind="ExternalInput")
    w = nc.dram_tensor("w", [D, D], F16, kind="ExternalInput")
    yt = nc.dram_tensor("yt", [D, NPC], F32, kind="ExternalOutput")

    with tile.TileContext(nc) as tc:
        with (
            tc.tile_pool(name="xpool", bufs=1) as xpool,
            tc.tile_pool(name="wpool", bufs=1) as wpool,
            tc.tile_pool(name="opool", bufs=2) as opool,
            tc.tile_pool(name="psum", bufs=1, space="PSUM") as ppool,
        ):
            # ---- PE warmup: ~3.5us of tiny matmuls on zeros so HAM is at
            # K=8/8 before real work arrives. Output goes to the ps0 slot,
            # which the first real accumulation then reuses (PE is FIFO).
            warm = xpool.tile([128, 64], F16, name="warm", tag="warm")
            nc.gpsimd.memset(warm[:], 0)
            wps = ppool.tile([128, 64], F32, name="wps", tag="ps0")
            for _ in range(48):
                nc.tensor.matmul(wps[0:64, :], warm[:], warm[:])

            # ---- input loads: x j-chunks on the Sync ring (split in two so
            # the first matmuls start sooner), w chunks on the Scalar ring.
            xj = []
            for j in range(JC):
                xtile = xpool.tile([128, KC * 512], F16, name=f"xj{j}", tag=f"x{j}")
                src = xt[:, j * 512 : (j + 1) * 512]
                half = (KC // 2) * 128  # 512 DRAM rows = 4 k-chunks
                nc.sync.dma_start(
                    xtile[:, : KC // 2 * 512].rearrange("p (k n) -> p k n", n=512),
                    src[:half, :].rearrange("(k p) n -> p k n", p=128),
                )
                nc.sync.dma_start(
                    xtile[:, KC // 2 * 512 :].rearrange("p (k n) -> p k n", n=512),
                    src[half:, :].rearrange("(k p) n -> p k n", p=128),
                )
                xj.append(xtile)
            wk = []
            for k in range(KC):
                wt = wpool.tile([128, D], F16, name=f"wk{k}", tag=f"w{k}")
                nc.scalar.dma_start(wt[:], w[k * 128 : (k + 1) * 128, :])
                wk.append(wt)

            # ---- main loop: j outer, accumulate over k into 8 PSUM banks
            # (one per m), drain while the next j computes.
            for j in range(JC):
                psums = [
                    ppool.tile([128, 512], F32, name=f"ps{m}", tag=f"ps{m}")
                    for m in range(MC)
                ]
                for k in range(KC):
                    rhs = xj[j][:, k * 512 : (k + 1) * 512]
                    for m in range(MC):
                        nc.tensor.matmul(
                            psums[m][:],
                            wk[k][:, m * 128 : (m + 1) * 128],
                            rhs,
                            start=(k == 0),
                            stop=(k == KC - 1),
                        )
                for m in range(MC):
                    osb = opool.tile(
                        [128, 512], F32, name=f"osb{m}", tag=f"osb{m}"
                    )
                    if m % 2 == 0:
                        nc.vector.tensor_copy(osb[:], psums[m][:])
                    else:
                        nc.scalar.copy(osb[:], psums[m][:])
                    nc.sync.dma_start(
                        yt[m * 128 : (m + 1) * 128, j * 512 : (j + 1) * 512], osb[:]
                    )

    nc.compile()
    _NC_CACHE = nc
    return nc


def _run(x: np.ndarray, cores: np.ndarray, trace: bool = False):
    from concourse.bass_utils import run_bass_kernel_spmd

    W = build_w(cores)
    wb = np.ascontiguousarray(W.astype(np.float32).astype(np.float16))

    xb = x.astype(np.float16)
    in_maps = []
    for c in range(N_CORES):
        xt_c = np.ascontiguousarray(xb[c * NPC : (c + 1) * NPC, :].T)
        in_maps.append({"xt": xt_c, "w": wb})

    nc = _build_bass()
    res = run_bass_kernel_spmd(
        nc, in_maps, core_ids=list(range(N_CORES)), trace=trace
    )

    y = np.empty((B, D), dtype=np.float32)
    for c in range(N_CORES):
        y[c * NPC : (c + 1) * NPC, :] = res.results[c]["yt"].T
    return y, res


def kernel(x: np.ndarray, cores: np.ndarray) -> np.ndarray:
    y, _ = _run(x, cores, trace=False)
    return y
